# revision 33
# baseline (speedup 1.0000x reference)
"""Trainium2 Bass kernel for nn_Attention_noZeromap (pooled-attention block).

Contract: kernel(**inputs) takes the FULL inputs from setup_inputs() as numpy
arrays and returns the FULL [8,128,128,128] fp32 output. The batch (B=8) is
data-parallel across the 8 NeuronCores (one sample per core); all params are
folded on host and replicated.

Per-sample dataflow (C-on-partitions primary layout, free = h*128+w):
  stage 1:
    xb    = bf16(x) loaded via one gpsimd casting DMA
    rstd  = (mean_c(xb^2)+eps)^-1/2 (ones-matmul broadcast + Ln/Exp on ACT;
            mean subtraction folded exactly into row-centered qkv weights)
    y     = xb*rstd
    k0,v0 = W2k/W2v @ y -> evacuated to guarded fp8e4m3 tiles
    q     = pooled only: in-place H-tree of y + edge rows -> tiny matmuls
    kd,vd = depthwise 3x3 entirely on PE: 5 fp8 DoubleRow diag-matmul passes
            (4 shifted tap pairs + 1 zero-padded pair) accumulated in PSUM;
            w-border columns recomputed exactly on DVE (taps x64 on host,
            1/64 folded into proj weights; kd unscaled - l2n is scale-inv)
    k1    = l2n(max_h kd) (in-place tree); A1 = softmax(q1^T k1 * temp1)
    vdT   = [w,(h,c)] via 4 sliced SBUF->SBUF xbar-transpose DMAs
    attT  = A1^T @ vdT = [v,(h,c)]; same transpose -> o1att [c,(h,v)]
    out1  = proj@o1att + xb via identity-matmul residual fold; evac overwrites
            the xb tile in place (bf16)
  stage 2: same skeleton; q2/k2 pooled over channels via an 18-row tap
    projection matmul bounced through DRAM (only remaining DRAM scratch); the
    v dwconv combine writes (w,h)-major so the transpose yields vd2T=[h,(w,c)];
    apply gives o2T=[g,(w,c)] -> o2=[c,(w,g)]; final proj reads o2 through a
    permuted rhs AP so PSUM comes out (h,w)-major, accumulates the out1
    residual via identity fold, and streams straight to HBM.
"""

import numpy as np

import concourse.bass as bass
import concourse.mybir as mybir
from concourse import bass_utils
from concourse.tile import ScopedClock, TileContext

# ---------------------------------------------------------------------------
# Walrus in this environment rejects >1 sem-wait on a CTRL (Drain)
# instruction; TileContext's tail drain aggregates one wait per active
# processor. Spread the excess over no-op carriers on the same engine.


def _drain_and_barrier_split(self, tick_clock, wait_clock):
    drain_inst = self.nc.sync.drain()
    wait_clock.add_sem_waits(
        drain_inst.ins, ScopedClock({None: tick_clock.global_clock})
    )
    si = drain_inst.ins.sync_info
    if si is not None and si.on_wait and len(si.on_wait) > 1:
        waits = list(si.on_wait)
        si.on_wait = waits[:1]
        for w in waits[1:]:
            nop = self.nc.sync.nop(nofuse=True)
            nop.ins.sync_info = mybir.SyncInfo(on_wait=[w], on_update=[])
    self.nc.all_engine_barrier()
    assert self.sems is not None
    popped = self.nc._tile_sem_poison_stack.pop()
    assert popped is self._sem_poison
    self.nc.clear_and_free_semaphores(list(self.sems.allocated().values()))
    self.nc.all_engine_barrier()


TileContext._drain_and_barrier = _drain_and_barrier_split


_WAIT_LIMIT = 1


def _split_excess_waits(raw: bytes) -> bytes:
    """Same workaround at the whole-program level: walrus in this env only
    accepts one sem-wait per instruction, but the Tile scheduler can attach
    several. Hoist the extras onto NoOp carriers just before the instruction
    on the same engine (FIFO streams, no dynamic control flow here)."""
    import json

    m = json.loads(raw)
    ctr = 0
    for fn in m["functions"]:
        for blk in fn["blocks"]:
            out = []
            for inst in blk["instructions"]:
                si = inst.get("sync_info")
                ow = (si or {}).get("on_wait") or []
                if len(ow) > _WAIT_LIMIT:
                    keep, extra = ow[-_WAIT_LIMIT:], ow[: -_WAIT_LIMIT]
                    for w in extra:
                        ctr += 1
                        out.append({
                            "name": f"I-wsplit-{ctr}",
                            "opcode": "NoOp",
                            "engine": inst["engine"],
                            "ins": [], "outs": [],
                            "sync_info": {"on_update": [], "on_wait": [w]},
                            "debug": inst.get("debug", 0),
                        })
                    si["on_wait"] = keep
                out.append(inst)
            blk["instructions"] = out
    return json.dumps(m).encode()
# ---------------------------------------------------------------------------

P = 128
C = 128
H = 128
W = 128
HW = H * W
CHUNK = 512
NCH = HW // CHUNK
GU = 192  # guard elems each side of fp8 dwconv inputs (zeros); taps reach 129
D0 = GU
EPS_LN = 1e-5
WS = 64.0  # fp8 tap weight scale; 1/WS folded into proj weights
BF = mybir.dt.bfloat16
F32 = mybir.dt.float32
FP8 = mybir.dt.float8e4
AX = mybir.AxisListType
ALU = mybir.AluOpType
ACTF = mybir.ActivationFunctionType
DR = mybir.MatmulPerfMode.DoubleRow

# tap order t = (dh+1)*3 + (dw+1); offset = dh*128 + dw
TAP_OFF = [(t // 3 - 1) * 128 + (t % 3 - 1) for t in range(9)]
# DoubleRow pairs of tap indices; tap 4 (dh=dw=0) rides a trailing normal
# fp8 matmul so each accumulation group ends in non-DR mode (ending the PE
# stream on a DoubleRow Matmult kills the NEFF at the tail drain).
DR_PAIRS = [(0, 6), (1, 7), (2, 8), (3, 5)]
SINGLE_TAP = 4


def _pair_offs(pi):
    t1, t2 = DR_PAIRS[pi]
    return TAP_OFF[t1], TAP_OFF[t2]


def _host_consts(inputs):
    """Fold params on host."""
    f = lambda k: np.asarray(inputs[k], np.float32)
    ln_w, ln_b = f("ln_w"), f("ln_b")
    qkv_w = f("qkv_w")[:, :, 0, 0]
    qkv_b = f("qkv_b")
    dw_w = f("dw_w")[:, 0]  # [3C,3,3]
    dw_b = f("dw_b")
    proj_w = f("proj_w")[:, :, 0, 0]
    proj_b = f("proj_b")

    assert (np.all(qkv_b == 0) and np.all(dw_b == 0) and np.all(ln_b == 0)
            and np.all(proj_b == 0)), (
        "nonzero qkv_b/dw_b/ln_b/proj_b not supported by the folded paths"
    )

    Wg = qkv_w * ln_w[None, :]  # ln scale folded
    Wq, Wk, Wv = Wg[:C], Wg[C : 2 * C], Wg[2 * C :]
    # exact mean-subtraction fold
    W2q = Wq - Wq.mean(axis=1, keepdims=True)
    W2k = Wk - Wk.mean(axis=1, keepdims=True)
    W2v = Wv - Wv.mean(axis=1, keepdims=True)

    Kq = dw_w[:C].reshape(C, 9)
    Kk = dw_w[C : 2 * C].reshape(C, 9)
    Kv = dw_w[2 * C :].reshape(C, 9)

    PQK = np.zeros((C, 18), np.float32)
    for t in range(9):
        PQK[:, t] = (W2q.T @ Kq[:, t]) / C
        PQK[:, 9 + t] = (W2k.T @ Kk[:, t]) / C

    # stage-1 pooled-q combination, per dw in {-1,0,1}:
    #   T_dw = AQ[:,dw]*S_q + EQ0[:,dw]*E0 + EQ127[:,dw]*E127   (EQ* negated)
    AQ = np.stack([Kq[:, dw] + Kq[:, 3 + dw] + Kq[:, 6 + dw] for dw in range(3)], 1)
    EQ0 = -Kq[:, 6:9]  # dh=+1 loses the h=0 row
    EQ127 = -Kq[:, 0:3]  # dh=-1 loses the h=127 row

    def dr_weights(K):
        w = np.zeros((C, len(DR_PAIRS), 2, C), np.float32)
        for pi, (t1, t2) in enumerate(DR_PAIRS):
            w[:, pi, 0, :] = np.diag(K[:, t1] * WS)
            w[:, pi, 1, :] = np.diag(K[:, t2] * WS)
        return w.reshape(C, len(DR_PAIRS) * 2 * C)

    def single_weights(K):
        return np.diag(K[:, SINGLE_TAP] * WS).astype(np.float32)

    return {
        "wq_lhsT": W2q.T.copy(),  # [C, O]
        "wk_lhsT": W2k.T.copy(),
        "wv_lhsT": W2v.T.copy(),
        "wp_lhsT": (proj_w.T / WS).copy(),  # 1/WS fold for the vd conv scale
        "ones": np.ones((C, P), np.float32),
        "pqk_lhsT": PQK,
        "ident": np.eye(P, dtype=np.float32),
        "kq": Kq, "kk": Kk, "kv": Kv,
        "aq": AQ, "eq0": EQ0, "eq127": EQ127,
        "drk": dr_weights(Kk), "drv": dr_weights(Kv),
        "sgk": single_weights(Kk), "sgv": single_weights(Kv),
        "epsln": np.full((P, 1), EPS_LN, np.float32),
        "eps24": np.full((P, 1), 1e-24, np.float32),
        "temp1": float(f("temp1").reshape(-1)[0]),
        "temp2": float(f("temp2").reshape(-1)[0]),
    }


CONST_SPECS = {
    "wq_lhsT": ([C, P], BF), "wk_lhsT": ([C, P], BF), "wv_lhsT": ([C, P], BF),
    "wp_lhsT": ([C, P], BF), "ones": ([C, P], BF), "pqk_lhsT": ([C, 18], BF),
    "ident": ([P, P], BF),
    "kq": ([C, 9], F32), "kk": ([C, 9], F32), "kv": ([C, 9], F32),
    "aq": ([C, 3], F32), "eq0": ([C, 3], F32), "eq127": ([C, 3], F32),
    "drk": ([C, len(DR_PAIRS) * 2 * C], FP8),
    "drv": ([C, len(DR_PAIRS) * 2 * C], FP8),
    "sgk": ([C, C], FP8), "sgv": ([C, C], FP8),
    "epsln": ([P, 1], F32), "eps24": ([P, 1], F32),
}


STOP_AT = None
DBG_SKIP_MID = False
DBG_SKIP_BORDERS = False
DBG_SKIP_CONV_MM = ()
DBG_CONV_BLOCKS = 8
_CONV_N = [0]


def _stop(ctx, name, out_d, flush_ap=None):
    if STOP_AT == name:
        if flush_ap is not None:
            fp = ctx.psmall.tile([P, P], F32, tag="pss", name=f"flush_{name}")
            ctx.nc.tensor.matmul(fp[:], ctx.cst["ident"], flush_ap)
        with ctx.tc.tile_pool(name="stopz", bufs=1) as zp:
            z = zp.tile([P, CHUNK], F32, tag="z")
            ctx.nc.vector.memset(z[:], 0.0)
            for j in range(NCH):
                ctx.nc.sync.dma_start(out_d[:, j * CHUNK : (j + 1) * CHUNK], z[:])
        return True
    return False


class Ctx:
    def __init__(self, nc, tc, cst, dbg):
        self.nc = nc
        self.tc = tc
        self.cst = cst
        self.dbg = dbg
        self.chain = None   # 3 rotating [P,HW] bf16 slots
        self.f8 = None      # 2 rotating guarded fp8 slots
        self.dram = None
        self.psmall = None
        self.smalls = None

    def dump(self, name, ap):
        if name in self.dbg:
            self.nc.sync.dma_start(self.dbg[name][:], ap)

    def big(self, name):
        return self.chain.tile([P, HW], BF, tag="chain", name=name)

    def bigf8(self, name):
        t = self.f8.tile([P, GU + HW + GU], FP8, tag="f8", name=name)
        self.nc.vector.memset(t[:, :GU], 0.0)
        self.nc.vector.memset(t[:, GU + HW :], 0.0)
        return t

    def transpose_into(self, dst, src_ap, nsl=8):
        """dst[p, j, z] = src[z, j*128+p] via sliced SBUF->SBUF xbar DMAs.

        src_ap: [P, HW] bf16 AP; dst: [P, HW] tile (viewed [P, 128, 128]).
        """
        nc = self.nc
        view = dst[:].rearrange("p (j z) -> p j z", z=P)
        JG = HW // P
        step = JG // nsl
        for s in range(nsl):
            j0, j1 = s * step, (s + 1) * step
            nc.sync.dma_start_transpose(
                view[:, j0:j1, :], src_ap[:, j0 * P : j1 * P])


def emit_lnorm(ctx, src_ap, y, tag):
    """y = src * rstd, rstd = (mean_c(src^2)+eps)^-1/2; src bf16 [P,HW] AP."""
    nc, tc = ctx.nc, ctx.tc
    with tc.tile_pool(name=f"sq{tag}", bufs=2) as sqp, \
         tc.tile_pool(name=f"ss{tag}", bufs=2, space="PSUM") as ssp, \
         tc.tile_pool(name=f"rst{tag}", bufs=2) as rsp:
        for j in range(NCH):
            sl = slice(j * CHUNK, (j + 1) * CHUNK)
            sq = sqp.tile([P, CHUNK], BF, tag="sq", name=f"sq{j}")
            sq_eng = nc.gpsimd if j % 4 == 1 else nc.vector
            sq_eng.tensor_mul(sq[:], src_ap[:, sl], src_ap[:, sl])
            ss = ssp.tile([P, CHUNK], F32, tag="ss", name=f"ss{j}")
            nc.tensor.matmul(ss[:], ctx.cst["ones"], sq[:])
            rst = rsp.tile([P, CHUNK], BF, tag="rst", name=f"rst{j}")
            lnv = rsp.tile([P, CHUNK], F32, tag="lnv", name=f"lnv{j}")
            nc.scalar.activation(lnv[:], ss[:], ACTF.Ln,
                                 bias=ctx.cst["epsln"], scale=1.0 / C)
            nc.scalar.activation(rst[:], lnv[:], ACTF.Exp, scale=-0.5)
            y_eng = nc.gpsimd if j % 4 == 3 else nc.vector
            y_eng.tensor_mul(y[:, sl], src_ap[:, sl], rst[:])


def _evac(nc, dst_ap, ps_ap, j, act_every=2):
    """Copy psum->sbuf; chunk j goes to ACT when j % act_every == 0."""
    if j % act_every == 0:
        nc.scalar.activation(dst_ap, ps_ap, ACTF.Copy)
    else:
        nc.vector.tensor_copy(dst_ap, ps_ap)


def emit_gemm_f8(ctx, psum_pool, lhsT, y, out_f8, phase=0):
    """out_f8 guarded fp8 tile = lhsT.T @ y; evacs alternate ACT/DVE."""
    nc = ctx.nc
    for j in range(NCH):
        sl = slice(j * CHUNK, (j + 1) * CHUNK)
        osl = slice(D0 + j * CHUNK, D0 + (j + 1) * CHUNK)
        ps = psum_pool.tile([P, CHUNK], F32, tag="qkvp", name=f"qkvp{j}")
        nc.tensor.matmul(ps[:], lhsT, y[:, sl])
        _evac(nc, out_f8[:, osl], ps[:], j + phase, act_every=4)


def _pair_ap(src_f8, base, o1, o2):
    """[P, 2, 512] AP over guarded fp8 tile: pair dim strides (o2-o1)."""
    r = src_f8[:, 0 : 2 * CHUNK].rearrange("p (t n) -> p t n", n=CHUNK)
    v = r.ap
    v[1] = [o2 - o1, 2]
    v[2] = [1, CHUNK]
    r2 = r.copy()
    r2.ap = v
    r2.offset = base + o1
    return r2


def _conv_taps(nc, dr_lhsT, sg_lhsT, src_f8, ps, base):
    """Accumulate all 9 taps for one chunk into psum ps (trailing normal)."""
    for pi in range(len(DR_PAIRS)):
        o1, o2 = _pair_offs(pi)
        lhsT = dr_lhsT[:, pi * 2 * C : (pi + 1) * 2 * C].rearrange(
            "p (t m) -> p t m", t=2)
        rhs = _pair_ap(src_f8, base, o1, o2)
        nc.tensor.matmul(ps, lhsT, rhs, start=(pi == 0), stop=False,
                         perf_mode=DR)
    nc.tensor.matmul(ps, sg_lhsT, src_f8[:, base : base + CHUNK],
                     start=False, stop=True)


def _conv_border(nc, src_f8, taps, write_fn):
    """Recompute exact w-border columns; write_fn(w, col_ap) consumes a
    [P,128] h-indexed, WS-scaled column."""
    span = 127 * 128 + 1
    for w, dws in ((0, (0, 1)), (127, (-1, 0))):
        scr = write_fn.scr[:, 0:P]
        first = True
        for dw in dws:
            for dh in (-1, 0, 1):
                t = (dh + 1) * 3 + (dw + 1)
                s0 = D0 + dh * 128 + w + dw
                src = src_f8[:, s0 : s0 + span : 128]
                tap = taps[:, t : t + 1]
                if first:
                    nc.vector.tensor_scalar(scr, src, tap, None, ALU.mult)
                    first = False
                else:
                    nc.vector.scalar_tensor_tensor(
                        scr, src, tap, scr, ALU.mult, ALU.add)
        nc.vector.tensor_scalar(scr, scr, WS, None, ALU.mult)
        write_fn(w, scr)


def emit_dwconv_f8(ctx, src_f8, dr_lhsT, sg_lhsT, taps, out, wh_major=False):
    """Depthwise 3x3 of guarded fp8 src -> out bf16 [P,HW] tile, scaled by WS.

    All 9 taps ride PE: 4 fp8 DoubleRow diag-matmul passes + 1 trailing
    normal fp8 pass (group must not end in DR mode) into PSUM.
    wh_major=False: out (h,w)-major like src; True: (w,h)-major.
    """
    nc, tc = ctx.nc, ctx.tc
    out_wh = out[:].rearrange("p (w h) -> p h w", w=W)
    with tc.tile_pool(name="dwp", bufs=4, space="PSUM") as pwp:
        for blk in range(0, NCH, 4):
            pss = [pwp.tile([P, CHUNK], F32, tag="dwps", name=f"dw{blk}_{i}")
                   for i in range(4)]
            for bi in range(4):
                _conv_taps(nc, dr_lhsT, sg_lhsT, src_f8, pss[bi][:],
                           D0 + (blk + bi) * CHUNK)
            for bi in range(4):
                j = blk + bi
                if not wh_major:
                    osl = out[:, j * CHUNK : (j + 1) * CHUNK]
                else:
                    osl = out_wh[:, j * 4 : (j + 1) * 4, :]
                _evac(nc, osl, pss[bi][:], j)

    def wf(w, scr):
        if wh_major:
            nc.vector.tensor_copy(out[:, w * 128 : w * 128 + 128], scr)
        else:
            nc.vector.tensor_copy(out[:, w : w + span_ : 128], scr)
    span_ = 127 * 128 + 1
    _CONV_N[0] += 1
    wf.scr = ctx.smalls.tile([P, P], F32, tag="bscr", name=f"bscr{_CONV_N[0]}")
    _conv_border(nc, src_f8, taps, wf)


def emit_dwconv_f8_maxhalf(ctx, src_f8, dr_lhsT, sg_lhsT, taps, half):
    """kd path: dwconv + first max-tree level fused at the psum evac.

    half: [P, HW//2] bf16 tile; half[:, j*512+u] = max over the (j, j+16)
    chunk pair (h and h+64 rows). WS scale left in (l2n is scale-invariant).
    """
    nc, tc = ctx.nc, ctx.tc
    with tc.tile_pool(name="dwp", bufs=4, space="PSUM") as pwp:
        for blk in range(0, NCH // 2, 2):
            pss = []
            for bi in range(2):
                j = blk + bi
                pa = pwp.tile([P, CHUNK], F32, tag="dwps", name=f"kda{j}")
                pb = pwp.tile([P, CHUNK], F32, tag="dwps", name=f"kdb{j}")
                _conv_taps(nc, dr_lhsT, sg_lhsT, src_f8, pa[:],
                           D0 + j * CHUNK)
                _conv_taps(nc, dr_lhsT, sg_lhsT, src_f8, pb[:],
                           D0 + (j + 16) * CHUNK)
                pss.append((j, pa, pb))
            for j, pa, pb in pss:
                hsl = half[:, j * CHUNK : (j + 1) * CHUNK]
                nc.scalar.activation(hsl, pa[:], ACTF.Copy)
                nc.vector.tensor_tensor(hsl, hsl, pb[:], ALU.max)

    def wf(w, scr):
        fold = wf.fold[:, 0:64]
        nc.vector.tensor_tensor(fold, scr[:, 0:64], scr[:, 64:128], ALU.max)
        nc.vector.tensor_copy(half[:, w : w + 63 * 128 + 1 : 128], fold)
    wf.scr = ctx.smalls.tile([P, P], F32, tag="bscr", name="bscr_kd")
    wf.fold = ctx.smalls.tile([P, 64], F32, tag="bfold", name="bfold_kd")
    _conv_border(nc, src_f8, taps, wf)


def emit_l2n_rows(ctx, src_ap, out_bf, tag):
    """out_bf [P,128] bf16 = rows of src l2-normalized over free."""
    nc = ctx.nc
    sm = ctx.smalls
    scr = sm.tile([P, P], F32, tag="l2scr", name=f"l2scr_{tag}")
    ss = sm.tile([P, 1], F32, tag="l2ss", name=f"l2ss_{tag}")
    nc.vector.tensor_mul(scr[:], src_ap, src_ap)
    nc.vector.tensor_reduce(ss[:], scr[:], AX.X, ALU.add)
    lnv = sm.tile([P, 1], F32, tag="l2ln", name=f"l2ln_{tag}")
    nc.scalar.activation(lnv[:], ss[:], ACTF.Ln, bias=ctx.cst["eps24"])
    inv = sm.tile([P, 1], F32, tag="l2i", name=f"l2i_{tag}")
    nc.scalar.activation(inv[:], lnv[:], ACTF.Exp, scale=-0.5)
    nc.vector.tensor_scalar(out_bf[:], src_ap, inv[:], None, ALU.mult)


def emit_softmax(ctx, logits_ps, temp, out_bf, tag):
    """out_bf [128,128] bf16 = softmax over free axis of logits_ps*temp."""
    nc = ctx.nc
    sm = ctx.smalls
    mx = sm.tile([P, 1], F32, tag="smx", name=f"smx_{tag}")
    nc.vector.tensor_reduce(mx[:], logits_ps[:], AX.X, ALU.max)
    nb = sm.tile([P, 1], F32, tag="snb", name=f"snb_{tag}")
    nc.vector.tensor_scalar(nb[:], mx[:], -temp, None, ALU.mult)
    e = sm.tile([P, P], F32, tag="sexp", name=f"sexp_{tag}")
    nc.scalar.activation(e[:], logits_ps[:], ACTF.Exp, bias=nb[:], scale=temp)
    s = sm.tile([P, 1], F32, tag="ssum", name=f"ssum_{tag}")
    nc.vector.tensor_reduce(s[:], e[:], AX.X, ALU.add)
    r = sm.tile([P, 1], F32, tag="srcp", name=f"srcp_{tag}")
    nc.vector.reciprocal(r[:], s[:])
    nc.vector.tensor_scalar(out_bf[:], e[:], r[:], None, ALU.mult)


def emit_tree_reduce_inplace(ctx, buf_ap, out_ap, op):
    """Reduce [P,(h,w)] over h by pairwise halving, destroying buf_ap."""
    nc = ctx.nc
    n = HW // 2
    nc.vector.tensor_tensor(buf_ap[:, :n], buf_ap[:, :n], buf_ap[:, n : 2 * n], op)
    while n > 256:
        h = n // 2
        nc.vector.tensor_tensor(buf_ap[:, :h], buf_ap[:, :h], buf_ap[:, h : 2 * h], op)
        n = h
    nc.vector.tensor_tensor(out_ap, buf_ap[:, :128], buf_ap[:, 128:256], op)


def emit_tree_reduce_half(ctx, buf_ap, out_ap, op):
    """Same but starting from an [P, HW//2] tile."""
    nc = ctx.nc
    n = HW // 4
    nc.vector.tensor_tensor(buf_ap[:, :n], buf_ap[:, :n], buf_ap[:, n : 2 * n], op)
    while n > 256:
        h = n // 2
        nc.vector.tensor_tensor(buf_ap[:, :h], buf_ap[:, :h], buf_ap[:, h : 2 * h], op)
        n = h
    nc.vector.tensor_tensor(out_ap, buf_ap[:, :128], buf_ap[:, 128:256], op)


# ---------------------------------------------------------------------------


def build_nc(consts, debug=()):
    nc = bass.Bass("TRN2")
    x_d = nc.dram_tensor("x", [P, HW], F32, kind="ExternalInput")
    out_d = nc.dram_tensor("out", [P, HW], F32, kind="ExternalOutput")
    cst_d = {
        n: nc.dram_tensor(n, shp, dt, kind="ExternalInput")
        for n, (shp, dt) in CONST_SPECS.items()
    }
    dbg_d = {}
    for name, shp in debug:
        dbg_d[name] = nc.dram_tensor(f"dbg_{name}", shp, F32, kind="ExternalOutput")

    with TileContext(nc, pool_alloc_mode="queue") as tc:
        with tc.tile_pool(name="consts", bufs=1) as cp, \
             tc.tile_pool(name="smalls", bufs=1) as smalls, \
             tc.tile_pool(name="psmall", bufs=2, space="PSUM") as psmall, \
             tc.tile_pool(name="xbp", bufs=1) as xbp, \
             tc.tile_pool(name="kdhp", bufs=1) as kdhp, \
             tc.tile_pool(name="chain", bufs=3) as chain, \
             tc.tile_pool(name="f8p", bufs=2) as f8p, \
             tc.tile_pool(name="dram", bufs=1, space="DRAM") as dram:
            # x casting load first so its transfers lead the DMA queue
            xb = xbp.tile([P, HW], BF, tag="xb", name="xb")
            for si in range(8):
                sl = slice(si * (HW // 8), (si + 1) * (HW // 8))
                nc.gpsimd.dma_start(xb[:, sl], x_d[:, sl])
            cst = {}
            for n, (shp, dt) in CONST_SPECS.items():
                t = cp.tile(shp, dt, tag=n, name=n)
                nc.sync.dma_start(t[:], cst_d[n][:])
                cst[n] = t[:]
            ctx = Ctx(nc, tc, cst, dbg_d)
            ctx.chain = chain
            ctx.f8 = f8p
            ctx.dram = dram
            ctx.psmall = psmall
            ctx.smalls = smalls
            ctx.xb = xb
            ctx.kdh = kdhp.tile([P, HW // 2], BF, tag="kdh", name="kdh")
            _emit_model(ctx, x_d, out_d, consts)
    orig_to_json = nc.to_json_bytes
    nc.to_json_bytes = lambda: _split_excess_waits(orig_to_json())
    return nc


def _emit_model(ctx, x_d, out_d, consts):
    nc, tc = ctx.nc, ctx.tc
    cst = ctx.cst
    sm = ctx.smalls

    # ================= STAGE 1 =================
    xb = ctx.xb  # loaded in build_nc before the consts; doubles as out1

    y = ctx.big("y")
    emit_lnorm(ctx, xb[:], y, "s1")
    ctx.dump("y1", y[:])
    if _stop(ctx, "lnorm", out_d):
        return

    k0 = ctx.bigf8("k0")
    v0 = ctx.bigf8("v0")
    with tc.tile_pool(name="qkvps", bufs=2, space="PSUM") as qps:
        emit_gemm_f8(ctx, qps, cst["wk_lhsT"], y, k0, 0)
        emit_gemm_f8(ctx, qps, cst["wv_lhsT"], y, v0, 1)
    ctx.dump("k0", k0[:, D0 : D0 + HW])
    if _stop(ctx, "qkv", out_d):
        return

    # ---- pooled q path (consumes then destroys y) ----
    sq_ps = ctx.psmall.tile([P, 384], F32, tag="pss", name="sq_ps")
    nc.tensor.matmul(sq_ps[:, 128:256], cst["wq_lhsT"], y[:, 0:128])
    nc.tensor.matmul(sq_ps[:, 256:384], cst["wq_lhsT"], y[:, HW - 128 : HW])
    yh = sm.tile([P, P], BF, tag="yh")
    emit_tree_reduce_inplace(ctx, y[:], yh[:], ALU.add)
    nc.tensor.matmul(sq_ps[:, 0:128], cst["wq_lhsT"], yh[:])
    tg = sm.tile([P, 3 * 132], F32, tag="tg")
    nc.vector.memset(tg[:], 0.0)
    for dwi in range(3):
        tsl = tg[:, dwi * 132 + 1 : dwi * 132 + 129]
        nc.vector.tensor_scalar(
            tsl, sq_ps[:, 0:128], cst["aq"][:, dwi : dwi + 1], None, ALU.mult)
        nc.vector.scalar_tensor_tensor(
            tsl, sq_ps[:, 128:256], cst["eq0"][:, dwi : dwi + 1],
            tsl, ALU.mult, ALU.add)
        nc.vector.scalar_tensor_tensor(
            tsl, sq_ps[:, 256:384], cst["eq127"][:, dwi : dwi + 1],
            tsl, ALU.mult, ALU.add)
    q1pre = sm.tile([P, P], F32, tag="q1pre")
    nc.vector.tensor_add(q1pre[:], tg[:, 0:128], tg[:, 132 + 1 : 132 + 129])
    nc.vector.tensor_add(q1pre[:], q1pre[:], tg[:, 2 * 132 + 2 : 2 * 132 + 130])
    ctx.dump("q1pre", q1pre[:])
    q1 = sm.tile([P, P], BF, tag="q1")
    emit_l2n_rows(ctx, q1pre[:], q1, "q1")
    if _stop(ctx, "pooledq", out_d):
        return

    # ---- k + v dwconvs back to back so the PE stream never stalls on the
    # pooled-path smalls (which wait on DVE/ACT/DMA) ----
    kdh = ctx.kdh
    emit_dwconv_f8_maxhalf(ctx, k0, cst["drk"], cst["sgk"], cst["kk"], kdh)
    vd = ctx.big("vd")
    emit_dwconv_f8(ctx, v0, cst["drv"], cst["sgv"], cst["kv"], vd)
    ctx.dump("vd", vd[:])
    vdt = ctx.big("vdT")
    ctx.transpose_into(vdt, vd[:])

    kmax = sm.tile([P, P], BF, tag="kmax")
    emit_tree_reduce_half(ctx, kdh[:], kmax[:], ALU.max)
    kmaxf = sm.tile([P, P], F32, tag="kmaxf")
    nc.vector.tensor_copy(kmaxf[:], kmax[:])
    k1 = sm.tile([P, P], BF, tag="k1")
    emit_l2n_rows(ctx, kmaxf[:], k1, "k1")

    lg_ps = ctx.psmall.tile([P, P], F32, tag="pss", name="lg_ps")
    nc.tensor.matmul(lg_ps[:], q1[:], k1[:])
    a1 = sm.tile([P, P], BF, tag="a1")
    emit_softmax(ctx, lg_ps, consts["temp1"], a1, "a1")
    ctx.dump("a1", a1[:])
    if _stop(ctx, "a1", out_d):
        return

    # apply directly into o1att [c,(h,v)] via per-h block-lhsT matmuls
    o1att = ctx.big("o1att")  # (y dead)
    with tc.tile_pool(name="apps", bufs=2, space="PSUM") as aps:
        for j in range(NCH):
            ps = aps.tile([P, CHUNK], F32, tag="apps", name=f"ap{j}")
            for hi in range(4):
                h = j * 4 + hi
                nc.tensor.matmul(ps[:, hi * P : (hi + 1) * P],
                                 vdt[:, h * P : (h + 1) * P], a1[:])
            _evac(nc, o1att[:, j * CHUNK : (j + 1) * CHUNK], ps[:], j)
        ctx.dump("o1att", o1att[:])
        if _stop(ctx, "o1att", out_d, o1att[:, HW - P : HW]):
            return

        # proj + xb residual (identity fold) -> out1 overwrites xb in place
        prj = tc.tile_pool(name="prj1", bufs=2, space="PSUM")
        prjp = prj.__enter__()
        for j in range(NCH):
            sl = slice(j * CHUNK, (j + 1) * CHUNK)
            ps = prjp.tile([P, CHUNK], F32, tag="prj1", name=f"pj{j}")
            nc.tensor.matmul(ps[:], cst["wp_lhsT"], o1att[:, sl],
                             start=True, stop=False)
            nc.tensor.matmul(ps[:], cst["ident"], xb[:, sl],
                             start=False, stop=True)
            _evac(nc, xb[:, sl], ps[:], j)
        prj.__exit__(None, None, None)
    out1 = xb
    ctx.dump("out1", out1[:])
    if _stop(ctx, "out1", out_d):
        return
    _emit_stage2(ctx, out1, out_d, consts)


def _emit_stage2(ctx, out1, out_d, consts):
    nc, tc = ctx.nc, ctx.tc
    cst = ctx.cst
    sm = ctx.smalls

    y = ctx.big("y2")  # (vd dead)
    emit_lnorm(ctx, out1[:], y, "s2")
    ctx.dump("y2", y[:])
    if _stop(ctx, "lnorm2", out_d):
        return

    v0 = ctx.bigf8("v02")
    mscr = ctx.dram.tile([18, GU + HW + GU], BF, tag="mscr", name="mscr")
    with tc.tile_pool(name="qkvps2", bufs=2, space="PSUM") as qps, \
         tc.tile_pool(name="mstg", bufs=1) as mstg, \
         tc.tile_pool(name="mzero", bufs=1) as mzp:
        emit_gemm_f8(ctx, qps, cst["wv_lhsT"], y, v0, 0)
        # 18-row pooled q2/k2 tap projections -> DRAM (repartition bounce)
        mz = mzp.tile([18, GU], BF, tag="mz", name="mz")
        nc.vector.memset(mz[:], 0.0)
        nc.sync.dma_start(mscr[:, :GU], mz[:])
        nc.sync.dma_start(mscr[:, GU + HW :], mz[:])
        for blk in range(0, NCH, 4):
            stg = mstg.tile([18, 4 * CHUNK], BF, tag="mstg", name=f"mst{blk}")
            for bi in range(4):
                j = blk + bi
                ps = qps.tile([18, CHUNK], F32, tag="mps", name=f"mps{j}")
                nc.tensor.matmul(ps[:], cst["pqk_lhsT"], y[:, j * CHUNK : (j + 1) * CHUNK])
                _evac(nc, stg[:, bi * CHUNK : (bi + 1) * CHUNK], ps[:], j + 1, act_every=1000)
            nc.sync.dma_start(
                mscr[:, D0 + blk * CHUNK : D0 + (blk + 4) * CHUNK], stg[:])

    # ---- v path early: conv + transpose while the m bounce flies ----
    vd = ctx.big("vd2")
    emit_dwconv_f8(ctx, v0, cst["drv"], cst["sgv"], cst["kv"], vd,
                   wh_major=True)
    vdt = ctx.big("vd2T")
    ctx.transpose_into(vdt, vd[:])

    # repartition rows via DRAM; tap shifts folded into the read offsets
    qt = sm.tile([P, 9 * P], BF, tag="qtiles")
    kt = sm.tile([P, 9 * P], BF, tag="ktiles")
    for t in range(9):
        off = D0 + TAP_OFF[t]
        nc.sync.dma_start(
            qt[:, t * P : (t + 1) * P],
            mscr[t, off : off + HW].rearrange("(h w) -> h w", h=P))
        nc.sync.dma_start(
            kt[:, t * P : (t + 1) * P],
            mscr[9 + t, off : off + HW].rearrange("(h w) -> h w", h=P))
    for nm in ("q2", "k2"):
        tt = qt if nm == "q2" else kt
        acc = sm.tile([P, P], F32, tag=f"{nm}pre", name=f"{nm}pre")
        nc.vector.tensor_add(acc[:], tt[:, 0:P], tt[:, P : 2 * P])
        for t in range(2, 9):
            nc.vector.tensor_add(acc[:], acc[:], tt[:, t * P : (t + 1) * P])
        # exact w-border columns (dw=+-1 tiles wrapped)
        for w, bad_dw in ((0, 0), (127, 2)):
            first = True
            for t in range(9):
                if t % 3 == bad_dw:
                    continue
                src = tt[:, t * P + w : t * P + w + 1]
                if first:
                    nc.vector.tensor_copy(acc[:, w : w + 1], src)
                    first = False
                else:
                    nc.vector.tensor_add(acc[:, w : w + 1], acc[:, w : w + 1], src)
        ctx.dump(f"{nm}pre", acc[:])
        nbf = sm.tile([P, P], BF, tag=nm, name=nm)
        emit_l2n_rows(ctx, acc[:], nbf, nm)
        pst = ctx.psmall.tile([P, P], BF, tag="pss", name=f"{nm}tp")
        nc.tensor.transpose(pst[:], nbf[:], cst["ident"])
        ntp = sm.tile([P, P], BF, tag=f"{nm}T", name=f"{nm}T")
        nc.vector.tensor_copy(ntp[:], pst[:])
        if nm == "q2":
            q2t = ntp
        else:
            k2t = ntp

    lg_ps = ctx.psmall.tile([P, P], F32, tag="pss", name="lg_ps2")
    nc.tensor.matmul(lg_ps[:], q2t[:], k2t[:])
    a2 = sm.tile([P, P], BF, tag="a2")
    emit_softmax(ctx, lg_ps, consts["temp2"], a2, "a2")
    ctx.dump("a2", a2[:])
    if _stop(ctx, "a2", out_d):
        return

    # apply directly into o2 [c,(g,w)] ((h,w)-major!) via per-w block-lhsT
    # matmuls; the strided evac scatters each w column so proj2 can stream
    # plain chunks right behind it.
    o2 = ctx.big("o2")  # (y2 dead)
    o2v = o2[:].rearrange("p (g w) -> p w g", w=W)
    with tc.tile_pool(name="aps2", bufs=2, space="PSUM") as aps, \
         tc.tile_pool(name="prj2", bufs=2, space="PSUM") as prjp, \
         tc.tile_pool(name="ost", bufs=3) as ostp:
        def apply_chunk(j):
            ps = aps.tile([P, CHUNK], F32, tag="aps2", name=f"a2p{j}")
            for wi in range(4):
                w = j * 4 + wi
                nc.tensor.matmul(ps[:, wi * P : (wi + 1) * P],
                                 vdt[:, w * P : (w + 1) * P], a2[:])
            _evac(nc, o2v[:, j * 4 : (j + 1) * 4, :], ps[:], j)

        def proj_chunk(j):
            sl = slice(j * CHUNK, (j + 1) * CHUNK)
            ps = prjp.tile([P, CHUNK], F32, tag="prj2", name=f"fp{j}")
            nc.tensor.matmul(ps[:], cst["wp_lhsT"], o2[:, sl],
                             start=True, stop=False)
            nc.tensor.matmul(ps[:], cst["ident"], out1[:, sl],
                             start=False, stop=True)
            st = ostp.tile([P, CHUNK], F32, tag="ost", name=f"st{j}")
            _evac(nc, st[:], ps[:], j)
            nc.sync.dma_start(out_d[:, sl], st[:])

        for j in range(NCH):
            apply_chunk(j)
            if j >= 1:
                proj_chunk(j - 1)
        proj_chunk(NCH - 1)


# ---------------------------------------------------------------------------


def kernel(**inputs):
    B = 8
    trace = bool(inputs.pop("_trace", False))
    x = np.asarray(inputs["x"], np.float32)
    consts = _host_consts(inputs)
    nc = build_nc(consts)

    import ml_dtypes
    const_arrays = {}
    for n, (shp, dt) in CONST_SPECS.items():
        a = np.asarray(consts[n], np.float32).reshape(shp)
        if dt == BF:
            a = a.astype(ml_dtypes.bfloat16)
        elif dt == FP8:
            a = a.astype(ml_dtypes.float8_e4m3fn)
        const_arrays[n] = a

    in_maps = []
    for b in range(B):
        mm = {"x": x[b].reshape(P, HW).copy()}
        mm.update(const_arrays)
        in_maps.append(mm)

    res = bass_utils.run_bass_kernel_spmd(nc, in_maps, core_ids=list(range(B)),
                                          trace=trace)
    if trace:
        print(f"HW exec time: {res.exec_time_ns} ns")
        if res.instructions_and_trace:
            print("trace:", res.instructions_and_trace[1])
    return np.stack([res.results[b]["out"].reshape(C, H, W) for b in range(B)])


def check_build():
    rng = np.random.default_rng(0)
    fake = {
        "x": rng.normal(size=(8, C, H, W)).astype(np.float32),
        "ln_w": np.ones(C, np.float32), "ln_b": np.zeros(C, np.float32),
        "qkv_w": rng.normal(size=(3 * C, C, 1, 1)).astype(np.float32) * 0.02,
        "qkv_b": np.zeros(3 * C, np.float32),
        "dw_w": rng.normal(size=(3 * C, 1, 3, 3)).astype(np.float32) * 0.02,
        "dw_b": np.zeros(3 * C, np.float32),
        "proj_w": rng.normal(size=(C, C, 1, 1)).astype(np.float32) * 0.02,
        "proj_b": np.zeros(C, np.float32),
        "temp1": np.ones((1, 1), np.float32),
        "temp2": np.ones((1, 1), np.float32),
    }
    build_nc(_host_consts(fake))
    print("build OK")


if __name__ == "__main__":
    check_build()
